# revision 21
# baseline (speedup 1.0000x reference)
"""Trainium2 Bass kernel for DomainCalibratedLoss (v3: int8 + Schraudolph/ACT
exp + PE weighted-reduce).

loss_i = lse_j(logw[d_i, j] + x[i, j]) - (logw[d_i, t_i] + x[i, t_i])
out    = sum_i(loss_i) / N

Device computes sum_i ln S_i with S_i = sum_j w[d_i, j] * exp(x_ij); the
target-score term is an O(N) host gather+sum (f64), subtracted in combine().

Design (vs the 128-170us ACT/DVE-tree baseline):
  * x ships as int8 (x*21 clipped) -> HALF the HBM traffic of bf16, the
    dominant cost. Tile-major DRAM layout gives 1MB contiguous transfers.
  * exp is computed per class-major tile two ways, balanced across engines:
    - DVE Schraudolph: t = round(x*(a/21) + b) as int16 IS the bf16 bit
      pattern of ~exp(x) (a = 128/ln2, b calibrated, zero-mean ln error).
      int8 input caps DVE at 2x mode -> ~1.13us per [*, 2048] op.
    - ACT true exp (scale=1/21, bf16 out) at 1x -> ~2.0us per op.
    23/64 of ops go to ACT (Bresenham pattern), balancing both at ~45us.
  * the weighted reduction sum_j w[d,j]*E[j] runs on the PE: rows are
    HOST-SORTED by domain into 512-row single-domain chunks; chunk i's
    matmul pair uses stationary lhsT = zeros except column i = w[d(chunk)],
    so chunk i's 512 sums land on PSUM partition i. All 128 chunks of a core
    accumulate into ONE [128, 512] PSUM bank = S for the whole core.
  * ACT does one Ln over that bank; DVE tensor_reduce -> [128, 1] out.

Layout per core: M = 65536 rows = 128 chunks of 512 = 32 tiles of 2048
= 16 DMA groups of 2 tiles. x0 [16*128, 4096] i8 (classes 0..127, group-major),
x1 [16*72, 4096] i8 (classes 128..199). wt0/wt1: per-chunk stationary weight
tables (mostly zeros, bf16), DMA'd once outside the pass loop (amortized).

Pad rows (domain tails + tail chunks) have x=0; the host replays their exact
device value (Schraudolph E0 or ACT's exp(0)=1 per class-half) and subtracts
n_pad * ln(S_pad) in combine().

This walrus caps every engine instruction at ONE sync wait (see
_prune_redundant_waits). Buffer-reuse deps are routed through the group
DMAs (which wait on the last matmul of the group 4 slots back); everything
else needs only its single producer wait after pruning.
"""

import math
from contextlib import ExitStack

import numpy as np

import concourse.bass as bass
import concourse.tile as tile
from concourse import mybir
from concourse.tile_rust import add_dep_helper
from concourse.bass_utils import run_bass_kernel_spmd

F32 = mybir.dt.float32
BF16 = mybir.dt.bfloat16
I16 = mybir.dt.int16
I8 = mybir.dt.int8
BF16_NP = mybir.dt.np(BF16)

N_TOTAL = 500000
N_CORES = 8
N_PER = N_TOTAL // N_CORES
C = 200  # classes
C0 = 128  # first class chunk (partitions of x0)
C1 = C - C0  # 72
D = 8  # domains
IGNORE = 255

CHUNK = 512  # rows per single-domain chunk == PSUM free dim
NCHUNK = 128  # chunks per core (== PSUM partitions)
M = CHUNK * NCHUNK  # 65536 rows per core (padded)
TILE_ROWS = 2048  # rows per compute tile = 4 chunks
CPT = TILE_ROWS // CHUNK  # 4
NTILE = M // TILE_ROWS  # 32
GROUP_TILES = 2  # compute tiles per DMA group
GROUP_ROWS = GROUP_TILES * TILE_ROWS  # 8192
NGROUP = NTILE // GROUP_TILES  # 8
X_CLIP = 6.0
S_INT8 = 21.0  # x quantization scale

N_OPS = 2 * NTILE  # exp ops per pass (x0 + x1 per tile)
ACT_OPS = 24  # of N_OPS, how many run on ACT (true exp) vs DVE (Schraudolph)


def _op_engine(o):
    """Bresenham assignment of op o (0..N_OPS-1) -> 'act' or 'dve'."""
    return "act" if ((o + 1) * ACT_OPS) // N_OPS > (o * ACT_OPS) // N_OPS else "dve"


# Schraudolph constants: bf16 bits(v) = 128*e + m  (s=0), v = 2^(e-127)*(1+m/128)
A_EXP = 128.0 / math.log(2.0)


def _calibrate_b():
    """Pick b minimizing the mean ln-error of round(a*x+b) -> bf16 bits."""
    u = np.linspace(16256.0, 16256.0 + 128.0, 20001)[:-1]
    target = (u - 16256.0) * (math.log(2.0) / 128.0)

    def mean_err(delta):
        t = np.rint(u + delta).astype(np.int64)
        e = t >> 7
        m = t & 127
        val = np.ldexp(1.0 + m / 128.0, e - 127)
        return float(np.mean(np.log(val) - target))

    lo, hi = -16.0, 4.0
    for _ in range(60):
        mid = 0.5 * (lo + hi)
        if mean_err(mid) > 0:
            hi = mid
        else:
            lo = mid
    return 16256.0 + 0.5 * (lo + hi)


B_EXP = _calibrate_b()


def _schraudolph_np(x):
    """Exact replay of the device DVE exp approx (f32 TS + RNE convert).
    x here is the PRE-SCALED float (i8 value / S_INT8 times S_INT8... pass
    the raw i8 value; scaling by A_EXP/S_INT8 happens inside like on device).
    """
    t = np.rint(
        np.asarray(x, np.float64) * np.float32(A_EXP / S_INT8) + np.float32(B_EXP)
    ).astype(np.int64)
    e = t >> 7
    m = t & 127
    return np.ldexp(1.0 + m / 128.0, e - 127)


def _prune_redundant_waits(nc):
    """Drop sync waits provably implied (transitively) by other waits."""
    f = nc.m.functions[0]
    insts = []
    for bb in f.blocks:
        for inst in bb.instructions:
            insts.append(inst)

    streams = {}
    pos = {}
    for inst in insts:
        eng = str(inst.engine)
        streams.setdefault(eng, []).append(inst)
        pos[inst.name] = (eng, len(streams[eng]) - 1)

    sem_updates = {}
    for inst in insts:
        si = inst.sync_info
        if si is None:
            continue
        for upd in si.on_update:
            if upd.sync_type != "semaphore" or upd.update_mode not in (
                "sem-inc",
                "sem-add-imm",
            ):
                continue
            lst = sem_updates.setdefault(upd.ant_name, [])
            prev = lst[-1][0] if lst else 0
            lst.append((prev + upd.update_value, inst.name))

    def satisfier(w):
        if w.sync_type != "semaphore" or w.wait_mode != "sem-ge-imm":
            return None
        lst = sem_updates.get(w.ant_name)
        if not lst:
            return None
        for cum, nm in lst:
            if cum >= w.wait_value:
                return nm
        return None

    vc = {nm: {} for nm in pos}

    def join(dst, src):
        changed = False
        for k, v in src.items():
            if dst.get(k, -1) < v:
                dst[k] = v
                changed = True
        return changed

    for _ in range(16):
        changed = False
        for eng, stream in streams.items():
            run = {}
            for i, inst in enumerate(stream):
                nm = inst.name
                si = inst.sync_info
                if si is not None:
                    for w in si.on_wait:
                        s = satisfier(w)
                        if s is None:
                            continue
                        sp, sidx = pos[s]
                        join(run, vc[s])
                        if run.get(sp, -1) < sidx:
                            run[sp] = sidx
                if join(vc[nm], run):
                    changed = True
                join(run, {eng: i})
        if not changed:
            break

    for eng, stream in streams.items():
        for i, inst in enumerate(stream):
            si = inst.sync_info
            if si is None or len(si.on_wait) <= 1:
                continue
            known = {}
            if i > 0:
                join(known, vc[stream[i - 1].name])
                join(known, {eng: i - 1})
            waits = list(si.on_wait)
            sats = [satisfier(w) for w in waits]
            keep = [True] * len(waits)
            for _trial in range(len(waits)):
                dropped_any = False
                for j in range(len(waits)):
                    if not keep[j] or sats[j] is None:
                        continue
                    cover = dict(known)
                    for k in range(len(waits)):
                        if k == j or not keep[k] or sats[k] is None:
                            continue
                        join(cover, vc[sats[k]])
                        skp, skidx = pos[sats[k]]
                        if cover.get(skp, -1) < skidx:
                            cover[skp] = skidx
                    sp, sidx = pos[sats[j]]
                    if cover.get(sp, -1) >= sidx:
                        keep[j] = False
                        dropped_any = True
                if not dropped_any:
                    break
            new_waits = [w for w, k in zip(waits, keep) if k]
            if len(new_waits) != len(waits):
                inst.sync_info = mybir.SyncInfo(
                    on_wait=new_waits, on_update=list(si.on_update)
                )


def build_program(n_per=N_PER, num_devices=N_CORES, passes=1, ablate=()):
    """Build the Bass/Tile program for one core's shard (M padded rows)."""
    assert n_per == N_PER
    do_dma = "dma" not in ablate
    do_exp = "exp" not in ablate
    do_pe = "pe" not in ablate

    nc = bass.Bass(
        "TRN2",
        target_bir_lowering=False,
        debug=False,
        num_devices=num_devices,
    )

    x0_d = nc.dram_tensor(
        "x0", [NGROUP * C0, GROUP_ROWS], I8, kind="ExternalInput"
    ).ap()
    x1_d = nc.dram_tensor(
        "x1", [NGROUP * C1, GROUP_ROWS], I8, kind="ExternalInput"
    ).ap()
    wt0_d = nc.dram_tensor("wt0", [C0, NCHUNK * 128], BF16, kind="ExternalInput").ap()
    wt1_d = nc.dram_tensor("wt1", [C1, NCHUNK * 128], BF16, kind="ExternalInput").ap()
    out_d = nc.dram_tensor("out", [128, 1], F32, kind="ExternalOutput").ap()

    with ExitStack() as ctx:
        tc = ctx.enter_context(tile.TileContext(nc))

        singles = ctx.enter_context(tc.tile_pool(name="singles", bufs=1))
        xp0 = ctx.enter_context(tc.tile_pool(name="xp0", bufs=4))
        xp1 = ctx.enter_context(tc.tile_pool(name="xp1", bufs=4))
        ep0 = ctx.enter_context(tc.tile_pool(name="ep0", bufs=4))
        ep1 = ctx.enter_context(tc.tile_pool(name="ep1", bufs=4))
        pp = ctx.enter_context(tc.tile_pool(name="pp", bufs=2, space="PSUM"))
        lnp = ctx.enter_context(tc.tile_pool(name="lnp", bufs=2))

        wt0_sb = singles.tile([C0, NCHUNK * 128], BF16)
        wt1_sb = singles.tile([C1, NCHUNK * 128], BF16)
        nc.sync.dma_start(out=wt0_sb[:], in_=wt0_d[:, :])
        nc.sync.dma_start(out=wt1_sb[:], in_=wt1_d[:, :])

        out_sb = singles.tile([128, 1], F32)
        nc.vector.memset(out_sb[:], 0.0)

        last_mm_of_group = [None] * (NGROUP * passes)

        for _pass in range(passes):
            psum_t = pp.tile([128, CHUNK], F32, tag="ps")
            mm_i = None
            for g in range(NGROUP):
                gg = _pass * NGROUP + g
                x0_g = xp0.tile([C0, GROUP_ROWS], I8, tag="x0")
                x1_g = xp1.tile([C1, GROUP_ROWS], I8, tag="x1")
                e0_g = ep0.tile([C0, GROUP_ROWS], BF16, tag="e0")
                e1_g = ep1.tile([C1, GROUP_ROWS], BF16, tag="e1")
                if do_dma:
                    d0 = nc.sync.dma_start(
                        out=x0_g[:], in_=x0_d[g * C0 : (g + 1) * C0, :]
                    )
                    d1 = nc.sync.dma_start(
                        out=x1_g[:], in_=x1_d[g * C1 : (g + 1) * C1, :]
                    )
                    # slot-reuse: group DMA must not overwrite x/E of group
                    # gg-2 until that group's last matmul retired (the MM
                    # transitively implies all its exp/TS readers).
                    if gg >= 4 and last_mm_of_group[gg - 4] is not None:
                        prev = last_mm_of_group[gg - 4]
                        add_dep_helper(d0.ins, prev.ins, True, "x0 reuse")
                        add_dep_helper(d1.ins, prev.ins, True, "x1 reuse")

                for kt in range(GROUP_TILES):
                    t = g * GROUP_TILES + kt
                    tsl = slice(kt * TILE_ROWS, (kt + 1) * TILE_ROWS)
                    if do_exp:
                        for part, (x_g, e_g) in enumerate(
                            ((x0_g, e0_g), (x1_g, e1_g))
                        ):
                            o = 2 * t + part
                            if _op_engine(o) == "act":
                                nc.scalar.activation(
                                    e_g[:, tsl],
                                    x_g[:, tsl],
                                    mybir.ActivationFunctionType.Exp,
                                    scale=1.0 / S_INT8,
                                )
                            else:
                                nc.vector.tensor_scalar(
                                    out=e_g[:, tsl].bitcast(I16),
                                    in0=x_g[:, tsl],
                                    scalar1=float(A_EXP / S_INT8),
                                    scalar2=float(B_EXP),
                                    op0=mybir.AluOpType.mult,
                                    op1=mybir.AluOpType.add,
                                )

                    if do_pe:
                        for k in range(CPT):
                            i = t * CPT + k  # chunk index == PSUM partition
                            csl = slice(
                                kt * TILE_ROWS + k * CHUNK,
                                kt * TILE_ROWS + (k + 1) * CHUNK,
                            )
                            wsl = slice(i * 128, (i + 1) * 128)
                            nc.tensor.matmul(
                                out=psum_t[:, :],
                                lhsT=wt0_sb[:, wsl],
                                rhs=e0_g[:, csl],
                                start=(i == 0),
                                stop=False,
                            )
                            mm_i = nc.tensor.matmul(
                                out=psum_t[:, :],
                                lhsT=wt1_sb[:, wsl],
                                rhs=e1_g[:, csl],
                                start=False,
                                stop=(i == NCHUNK - 1),
                            )
                last_mm_of_group[gg] = mm_i

            if do_pe:
                ln_t = lnp.tile([128, CHUNK], BF16, tag="ln")
                nc.scalar.activation(
                    ln_t[:],
                    psum_t[:],
                    mybir.ActivationFunctionType.Ln,
                )
                nc.vector.tensor_reduce(
                    out=out_sb[:, 0:1],
                    in_=ln_t[:],
                    axis=mybir.AxisListType.X,
                    op=mybir.AluOpType.add,
                )
            nc.sync.dma_start(out=out_d[:, :], in_=out_sb[:])

    _prune_redundant_waits(nc)

    violations = []
    f = nc.m.functions[0]
    for bb in f.blocks:
        for inst in bb.instructions:
            si = inst.sync_info
            if si is None:
                continue
            nm = type(inst).__name__
            if nm in (
                "InstDrain",
                "InstEventSemaphore",
                "InstUnconditionalBranch",
                "InstRegisterMove",
                "InstCall",
                "InstNoOp",
            ):
                continue
            if len(si.on_wait) > 1:
                violations.append(
                    (
                        inst.name,
                        nm,
                        str(inst.engine),
                        [(w.ant_name, w.wait_value) for w in si.on_wait],
                    )
                )
    nc._wait_violations = violations

    return (nc,)


def prep_inmaps(inputs, targets, domains, dcc_weights, n_cores, n_per):
    """Host-side prep: O(N) index work + O(N*C) layout/dtype baking."""
    assert n_per == N_PER and n_cores == N_CORES
    x = np.asarray(inputs, dtype=np.float32)
    targets = np.asarray(targets).astype(np.int64).reshape(-1)
    domains = np.asarray(domains).astype(np.int64).reshape(-1)
    dcc = np.asarray(dcc_weights, dtype=np.float32)
    n = x.shape[0]

    logw = np.full_like(dcc, -np.inf)
    np.log(dcc, out=logw, where=dcc > 0)

    # host-side target-score sum: sum_i x[i, t_i] + logw[d_i, t_i] (f64 exact)
    t_scores = x[np.arange(n), targets].astype(np.float64)
    t_scores += logw.astype(np.float64)[domains, targets]
    tsum = float(t_scores.sum())

    # sort rows by domain into 512-row single-domain chunks
    order = np.argsort(domains, kind="stable")
    counts = np.bincount(domains, minlength=D)
    aligned = ((counts + CHUNK - 1) // CHUNK) * CHUNK
    total_chunks = n_cores * NCHUNK
    assert aligned.sum() <= total_chunks * CHUNK, "domain counts exceed capacity"
    base = np.concatenate(([0], np.cumsum(aligned)[:-1]))
    starts = np.concatenate(([0], np.cumsum(counts)[:-1]))
    dom_sorted = domains[order]
    rank = np.arange(n) - np.repeat(starts, counts)
    pos = base[dom_sorted] + rank  # padded position of each sorted row

    chunk_dom = np.full(total_chunks, D - 1, dtype=np.int64)
    for dd in range(D):
        c0 = base[dd] // CHUNK
        c1 = c0 + aligned[dd] // CHUNK
        chunk_dom[c0:c1] = dd

    # padded, clipped, sorted x in int8 (pad rows = 0)
    xi = np.clip(np.rint(x * S_INT8), -X_CLIP * S_INT8, X_CLIP * S_INT8).astype(
        np.int8
    )
    Xp = np.zeros((total_chunks * CHUNK, C), dtype=np.int8)
    Xp[pos] = xi[order]

    # pad correction: pad row in chunk i contributes
    #   ln( e_lo(i) * sum_{j<128} w_bf[d,j] + e_hi(i) * sum_{j>=128} w_bf[d,j] )
    # where e_* is the engine-exact exp(0): Schraudolph E0 on DVE, 1.0 on ACT.
    w_bf = dcc.astype(BF16_NP)
    E0 = float(_schraudolph_np(np.float32(0.0)))
    W_lo = w_bf[:, :C0].astype(np.float64).sum(axis=1)
    W_hi = w_bf[:, C0:].astype(np.float64).sum(axis=1)
    # chunk i -> tile t = i//CPT -> ops (2t, 2t+1)
    i_arr = np.arange(NCHUNK)  # chunk within core (same for every core)
    t_arr = i_arr // CPT
    e_lo = np.array(
        [1.0 if _op_engine(2 * t) == "act" else E0 for t in t_arr]
    )
    e_hi = np.array(
        [1.0 if _op_engine(2 * t + 1) == "act" else E0 for t in t_arr]
    )
    n_pad_per_chunk = np.full(total_chunks, CHUNK, dtype=np.int64)
    real_per_chunk = np.bincount(pos // CHUNK, minlength=total_chunks)
    n_pad_per_chunk -= real_per_chunk
    S_pad = (
        np.tile(e_lo, n_cores) * W_lo[chunk_dom]
        + np.tile(e_hi, n_cores) * W_hi[chunk_dom]
    )
    correction = float((n_pad_per_chunk * np.log(S_pad)).sum())

    in_maps = []
    for c in range(n_cores):
        blk = Xp[c * M : (c + 1) * M]  # [M, 200] i8
        x0 = np.ascontiguousarray(
            blk[:, :C0].reshape(NGROUP, GROUP_ROWS, C0).transpose(0, 2, 1)
        ).reshape(NGROUP * C0, GROUP_ROWS)
        x1 = np.ascontiguousarray(
            blk[:, C0:].reshape(NGROUP, GROUP_ROWS, C1).transpose(0, 2, 1)
        ).reshape(NGROUP * C1, GROUP_ROWS)
        cd = chunk_dom[c * NCHUNK : (c + 1) * NCHUNK]
        wt0 = np.zeros((C0, NCHUNK * 128), dtype=BF16_NP)
        wt1 = np.zeros((C1, NCHUNK * 128), dtype=BF16_NP)
        idx = np.arange(NCHUNK)
        wt0[:, idx * 128 + idx] = w_bf[cd, :C0].T
        wt1[:, idx * 128 + idx] = w_bf[cd, C0:].T
        in_maps.append({"x0": x0, "x1": x1, "wt0": wt0, "wt1": wt1})

    aux = {"tsum": tsum, "correction": correction, "n": n}
    return in_maps, aux


def combine(results, aux):
    """Combine per-core [128, 1] ln-S partials minus host-side terms."""
    total = -aux["tsum"] - aux["correction"]
    for r in results:
        o = np.asarray(r["out"], dtype=np.float64)
        total += float(o[:, 0].sum())
    return np.float32(total / aux["n"])


_PROGRAM_CACHE = {}


def _get_program(n_per, n_cores):
    key = (n_per, n_cores)
    if key not in _PROGRAM_CACHE:
        _PROGRAM_CACHE[key] = build_program(n_per, n_cores)
    return _PROGRAM_CACHE[key]


LAST_RESULT = None


def run(inputs, targets, domains, dcc_weights, trace=False, tmpdir=None):
    global LAST_RESULT
    n = inputs.shape[0]
    assert n % N_CORES == 0
    n_per = n // N_CORES
    (nc,) = _get_program(n_per, N_CORES)
    in_maps, aux = prep_inmaps(
        inputs, targets, domains, dcc_weights, N_CORES, n_per
    )
    res = run_bass_kernel_spmd(
        nc, in_maps, core_ids=list(range(N_CORES)), trace=trace, tmpdir=tmpdir
    )
    LAST_RESULT = res
    return combine(res.results, aux)


def kernel(inputs, targets, domains, dcc_weights):
    targets = np.asarray(targets).reshape(-1)
    domains_a = np.asarray(domains).reshape(-1)
    counts = np.bincount(domains_a, minlength=D) if domains_a.size else np.zeros(D)
    aligned_total = (((counts + CHUNK - 1) // CHUNK) * CHUNK).sum()
    if (
        np.any((targets < 0) | (targets >= C))
        or np.asarray(inputs).shape[0] != N_TOTAL
        or np.any(domains_a < 0)
        or np.any(domains_a >= D)
        or aligned_total > N_CORES * NCHUNK * CHUNK
    ):
        # exact but slow host fallback (never hit for the spec'd inputs)
        x = np.asarray(inputs, dtype=np.float64)
        dcc = np.asarray(dcc_weights, dtype=np.float64)
        logw = np.where(dcc > 0, np.log(np.maximum(dcc, 1e-300)), -np.inf)
        scores = logw[domains_a] + x
        m = scores.max(axis=1)
        lse = m + np.log(np.exp(scores - m[:, None]).sum(axis=1))
        tgt = np.clip(targets, 0, C - 1)
        ts = scores[np.arange(x.shape[0]), tgt]
        valid = targets != IGNORE
        return np.float32(np.where(valid, lse - ts, 0.0).sum() / x.shape[0])
    return run(inputs, targets, domains, dcc_weights, trace=False)


# revision 22
# speedup vs baseline: 1.3465x; 1.3465x over previous
"""Trainium2 Bass kernel for DomainCalibratedLoss (v3: int8 + Schraudolph/ACT
exp + PE weighted-reduce).

loss_i = lse_j(logw[d_i, j] + x[i, j]) - (logw[d_i, t_i] + x[i, t_i])
out    = sum_i(loss_i) / N

Device computes sum_i ln S_i with S_i = sum_j w[d_i, j] * exp(x_ij); the
target-score term is an O(N) host gather+sum (f64), subtracted in combine().

Design (vs the 128-170us ACT/DVE-tree baseline):
  * x ships as int8 (x*21 clipped) -> HALF the HBM traffic of bf16, the
    dominant cost. Tile-major DRAM layout gives 1MB contiguous transfers.
  * exp is computed per class-major tile two ways, balanced across engines:
    - DVE Schraudolph: t = round(x*(a/21) + b) as int16 IS the bf16 bit
      pattern of ~exp(x) (a = 128/ln2, b calibrated, zero-mean ln error).
      int8 input caps DVE at 2x mode -> ~1.13us per [*, 2048] op.
    - ACT true exp (scale=1/21, bf16 out) at 1x -> ~2.0us per op.
    23/64 of ops go to ACT (Bresenham pattern), balancing both at ~45us.
  * the weighted reduction sum_j w[d,j]*E[j] runs on the PE: rows are
    HOST-SORTED by domain into 512-row single-domain chunks; chunk i's
    matmul pair uses stationary lhsT = zeros except column i = w[d(chunk)],
    so chunk i's 512 sums land on PSUM partition i. All 128 chunks of a core
    accumulate into ONE [128, 512] PSUM bank = S for the whole core.
  * ACT does one Ln over that bank; DVE tensor_reduce -> [128, 1] out.

Layout per core: M = 65536 rows = 128 chunks of 512 = 32 tiles of 2048
= 16 DMA groups of 2 tiles. x0 [16*128, 4096] i8 (classes 0..127, group-major),
x1 [16*72, 4096] i8 (classes 128..199). wt0/wt1: per-chunk stationary weight
tables (mostly zeros, bf16), DMA'd once outside the pass loop (amortized).

Pad rows (domain tails + tail chunks) have x=0; the host replays their exact
device value (Schraudolph E0 or ACT's exp(0)=1 per class-half) and subtracts
n_pad * ln(S_pad) in combine().

This walrus caps every engine instruction at ONE sync wait (see
_prune_redundant_waits). Buffer-reuse deps are routed through the group
DMAs (which wait on the last matmul of the group 4 slots back); everything
else needs only its single producer wait after pruning.
"""

import math
from contextlib import ExitStack

import numpy as np

import concourse.bass as bass
import concourse.tile as tile
from concourse import mybir
from concourse.tile_rust import add_dep_helper
from concourse.bass_utils import run_bass_kernel_spmd

F32 = mybir.dt.float32
BF16 = mybir.dt.bfloat16
I16 = mybir.dt.int16
I8 = mybir.dt.int8
BF16_NP = mybir.dt.np(BF16)

N_TOTAL = 500000
N_CORES = 8
N_PER = N_TOTAL // N_CORES
C = 200  # classes
C0 = 128  # first class chunk (partitions of x0)
C1 = C - C0  # 72
D = 8  # domains
IGNORE = 255

CHUNK = 512  # rows per single-domain chunk == PSUM free dim
NCHUNK = 128  # chunks per core (== PSUM partitions)
M = CHUNK * NCHUNK  # 65536 rows per core (padded)
TILE_ROWS = 2048  # rows per compute tile = 4 chunks
CPT = TILE_ROWS // CHUNK  # 4
NTILE = M // TILE_ROWS  # 32
GROUP_TILES = 2  # compute tiles per DMA group
GROUP_ROWS = GROUP_TILES * TILE_ROWS  # 8192
NGROUP = NTILE // GROUP_TILES  # 8
X_CLIP = 6.0
S_INT8 = 21.0  # x quantization scale

N_OPS = 2 * NTILE  # exp ops per pass (x0 + x1 per tile)
ACT_OPS = 23  # of N_OPS, how many run on ACT (true exp) vs DVE (Schraudolph)


def _op_engine(o):
    """Bresenham assignment of op o (0..N_OPS-1) -> 'act' or 'dve'."""
    return "act" if ((o + 1) * ACT_OPS) // N_OPS > (o * ACT_OPS) // N_OPS else "dve"


# Schraudolph constants: bf16 bits(v) = 128*e + m  (s=0), v = 2^(e-127)*(1+m/128)
A_EXP = 128.0 / math.log(2.0)


def _calibrate_b():
    """Pick b minimizing the mean ln-error of round(a*x+b) -> bf16 bits."""
    u = np.linspace(16256.0, 16256.0 + 128.0, 20001)[:-1]
    target = (u - 16256.0) * (math.log(2.0) / 128.0)

    def mean_err(delta):
        t = np.rint(u + delta).astype(np.int64)
        e = t >> 7
        m = t & 127
        val = np.ldexp(1.0 + m / 128.0, e - 127)
        return float(np.mean(np.log(val) - target))

    lo, hi = -16.0, 4.0
    for _ in range(60):
        mid = 0.5 * (lo + hi)
        if mean_err(mid) > 0:
            hi = mid
        else:
            lo = mid
    return 16256.0 + 0.5 * (lo + hi)


B_EXP = _calibrate_b()


def _schraudolph_np(x):
    """Exact replay of the device DVE exp approx (f32 TS + RNE convert).
    x here is the PRE-SCALED float (i8 value / S_INT8 times S_INT8... pass
    the raw i8 value; scaling by A_EXP/S_INT8 happens inside like on device).
    """
    t = np.rint(
        np.asarray(x, np.float64) * np.float32(A_EXP / S_INT8) + np.float32(B_EXP)
    ).astype(np.int64)
    e = t >> 7
    m = t & 127
    return np.ldexp(1.0 + m / 128.0, e - 127)


def _prune_redundant_waits(nc):
    """Drop sync waits provably implied (transitively) by other waits."""
    f = nc.m.functions[0]
    insts = []
    for bb in f.blocks:
        for inst in bb.instructions:
            insts.append(inst)

    streams = {}
    pos = {}
    for inst in insts:
        eng = str(inst.engine)
        streams.setdefault(eng, []).append(inst)
        pos[inst.name] = (eng, len(streams[eng]) - 1)

    sem_updates = {}
    for inst in insts:
        si = inst.sync_info
        if si is None:
            continue
        for upd in si.on_update:
            if upd.sync_type != "semaphore" or upd.update_mode not in (
                "sem-inc",
                "sem-add-imm",
            ):
                continue
            lst = sem_updates.setdefault(upd.ant_name, [])
            prev = lst[-1][0] if lst else 0
            lst.append((prev + upd.update_value, inst.name))

    def satisfier(w):
        if w.sync_type != "semaphore" or w.wait_mode != "sem-ge-imm":
            return None
        lst = sem_updates.get(w.ant_name)
        if not lst:
            return None
        for cum, nm in lst:
            if cum >= w.wait_value:
                return nm
        return None

    vc = {nm: {} for nm in pos}

    def join(dst, src):
        changed = False
        for k, v in src.items():
            if dst.get(k, -1) < v:
                dst[k] = v
                changed = True
        return changed

    for _ in range(16):
        changed = False
        for eng, stream in streams.items():
            run = {}
            for i, inst in enumerate(stream):
                nm = inst.name
                si = inst.sync_info
                if si is not None:
                    for w in si.on_wait:
                        s = satisfier(w)
                        if s is None:
                            continue
                        sp, sidx = pos[s]
                        join(run, vc[s])
                        if run.get(sp, -1) < sidx:
                            run[sp] = sidx
                if join(vc[nm], run):
                    changed = True
                join(run, {eng: i})
        if not changed:
            break

    for eng, stream in streams.items():
        for i, inst in enumerate(stream):
            si = inst.sync_info
            if si is None or len(si.on_wait) <= 1:
                continue
            known = {}
            if i > 0:
                join(known, vc[stream[i - 1].name])
                join(known, {eng: i - 1})
            waits = list(si.on_wait)
            sats = [satisfier(w) for w in waits]
            keep = [True] * len(waits)
            for _trial in range(len(waits)):
                dropped_any = False
                for j in range(len(waits)):
                    if not keep[j] or sats[j] is None:
                        continue
                    cover = dict(known)
                    for k in range(len(waits)):
                        if k == j or not keep[k] or sats[k] is None:
                            continue
                        join(cover, vc[sats[k]])
                        skp, skidx = pos[sats[k]]
                        if cover.get(skp, -1) < skidx:
                            cover[skp] = skidx
                    sp, sidx = pos[sats[j]]
                    if cover.get(sp, -1) >= sidx:
                        keep[j] = False
                        dropped_any = True
                if not dropped_any:
                    break
            new_waits = [w for w, k in zip(waits, keep) if k]
            if len(new_waits) != len(waits):
                inst.sync_info = mybir.SyncInfo(
                    on_wait=new_waits, on_update=list(si.on_update)
                )


def build_program(n_per=N_PER, num_devices=N_CORES, passes=1, ablate=()):
    """Build the Bass/Tile program for one core's shard (M padded rows)."""
    assert n_per == N_PER
    do_dma = "dma" not in ablate
    do_exp = "exp" not in ablate
    do_pe = "pe" not in ablate

    nc = bass.Bass(
        "TRN2",
        target_bir_lowering=False,
        debug=False,
        num_devices=num_devices,
    )

    x0_d = nc.dram_tensor(
        "x0", [NGROUP * C0, GROUP_ROWS], I8, kind="ExternalInput"
    ).ap()
    x1_d = nc.dram_tensor(
        "x1", [NGROUP * C1, GROUP_ROWS], I8, kind="ExternalInput"
    ).ap()
    wt0_d = nc.dram_tensor("wt0", [C0, NCHUNK * 128], BF16, kind="ExternalInput").ap()
    wt1_d = nc.dram_tensor("wt1", [C1, NCHUNK * 128], BF16, kind="ExternalInput").ap()
    out_d = nc.dram_tensor("out", [128, 1], F32, kind="ExternalOutput").ap()

    with ExitStack() as ctx:
        tc = ctx.enter_context(tile.TileContext(nc))

        singles = ctx.enter_context(tc.tile_pool(name="singles", bufs=1))
        xp0 = ctx.enter_context(tc.tile_pool(name="xp0", bufs=4))
        xp1 = ctx.enter_context(tc.tile_pool(name="xp1", bufs=4))
        ep0 = ctx.enter_context(tc.tile_pool(name="ep0", bufs=4))
        ep1 = ctx.enter_context(tc.tile_pool(name="ep1", bufs=4))
        pp = ctx.enter_context(tc.tile_pool(name="pp", bufs=2, space="PSUM"))
        lnp = ctx.enter_context(tc.tile_pool(name="lnp", bufs=2))

        wt0_sb = singles.tile([C0, NCHUNK * 128], BF16)
        wt1_sb = singles.tile([C1, NCHUNK * 128], BF16)
        nc.sync.dma_start(out=wt0_sb[:], in_=wt0_d[:, :])
        nc.sync.dma_start(out=wt1_sb[:], in_=wt1_d[:, :])

        out_sb = singles.tile([128, 1], F32)
        nc.vector.memset(out_sb[:], 0.0)

        last_mm_of_group = [None] * (NGROUP * passes)

        for _pass in range(passes):
            psum_t = pp.tile([128, CHUNK], F32, tag="ps")
            mm_i = None
            for g in range(NGROUP):
                gg = _pass * NGROUP + g
                x0_g = xp0.tile([C0, GROUP_ROWS], I8, tag="x0")
                x1_g = xp1.tile([C1, GROUP_ROWS], I8, tag="x1")
                e0_g = ep0.tile([C0, GROUP_ROWS], BF16, tag="e0")
                e1_g = ep1.tile([C1, GROUP_ROWS], BF16, tag="e1")
                if do_dma:
                    d0 = nc.sync.dma_start(
                        out=x0_g[:], in_=x0_d[g * C0 : (g + 1) * C0, :]
                    )
                    d1 = nc.sync.dma_start(
                        out=x1_g[:], in_=x1_d[g * C1 : (g + 1) * C1, :]
                    )
                    # slot-reuse: group DMA must not overwrite x/E of group
                    # gg-2 until that group's last matmul retired (the MM
                    # transitively implies all its exp/TS readers).
                    if gg >= 4 and last_mm_of_group[gg - 4] is not None:
                        prev = last_mm_of_group[gg - 4]
                        add_dep_helper(d0.ins, prev.ins, True, "x0 reuse")
                        add_dep_helper(d1.ins, prev.ins, True, "x1 reuse")

                for kt in range(GROUP_TILES):
                    t = g * GROUP_TILES + kt
                    tsl = slice(kt * TILE_ROWS, (kt + 1) * TILE_ROWS)
                    if do_exp:
                        for part, (x_g, e_g) in enumerate(
                            ((x0_g, e0_g), (x1_g, e1_g))
                        ):
                            o = 2 * t + part
                            if _op_engine(o) == "act":
                                nc.scalar.activation(
                                    e_g[:, tsl],
                                    x_g[:, tsl],
                                    mybir.ActivationFunctionType.Exp,
                                    scale=1.0 / S_INT8,
                                )
                            else:
                                nc.vector.tensor_scalar(
                                    out=e_g[:, tsl].bitcast(I16),
                                    in0=x_g[:, tsl],
                                    scalar1=float(A_EXP / S_INT8),
                                    scalar2=float(B_EXP),
                                    op0=mybir.AluOpType.mult,
                                    op1=mybir.AluOpType.add,
                                )

                    if do_pe:
                        for k in range(CPT):
                            i = t * CPT + k  # chunk index == PSUM partition
                            csl = slice(
                                kt * TILE_ROWS + k * CHUNK,
                                kt * TILE_ROWS + (k + 1) * CHUNK,
                            )
                            wsl = slice(i * 128, (i + 1) * 128)
                            nc.tensor.matmul(
                                out=psum_t[:, :],
                                lhsT=wt0_sb[:, wsl],
                                rhs=e0_g[:, csl],
                                start=(i == 0),
                                stop=False,
                            )
                            mm_i = nc.tensor.matmul(
                                out=psum_t[:, :],
                                lhsT=wt1_sb[:, wsl],
                                rhs=e1_g[:, csl],
                                start=False,
                                stop=(i == NCHUNK - 1),
                            )
                last_mm_of_group[gg] = mm_i

            if do_pe:
                ln_t = lnp.tile([128, CHUNK], BF16, tag="ln")
                nc.scalar.activation(
                    ln_t[:],
                    psum_t[:],
                    mybir.ActivationFunctionType.Ln,
                )
                nc.vector.tensor_reduce(
                    out=out_sb[:, 0:1],
                    in_=ln_t[:],
                    axis=mybir.AxisListType.X,
                    op=mybir.AluOpType.add,
                )
            nc.sync.dma_start(out=out_d[:, :], in_=out_sb[:])

    _prune_redundant_waits(nc)

    violations = []
    f = nc.m.functions[0]
    for bb in f.blocks:
        for inst in bb.instructions:
            si = inst.sync_info
            if si is None:
                continue
            nm = type(inst).__name__
            if nm in (
                "InstDrain",
                "InstEventSemaphore",
                "InstUnconditionalBranch",
                "InstRegisterMove",
                "InstCall",
                "InstNoOp",
            ):
                continue
            if len(si.on_wait) > 1:
                violations.append(
                    (
                        inst.name,
                        nm,
                        str(inst.engine),
                        [(w.ant_name, w.wait_value) for w in si.on_wait],
                    )
                )
    nc._wait_violations = violations

    return (nc,)


def prep_inmaps(inputs, targets, domains, dcc_weights, n_cores, n_per):
    """Host-side prep: O(N) index work + O(N*C) layout/dtype baking."""
    assert n_per == N_PER and n_cores == N_CORES
    x = np.asarray(inputs, dtype=np.float32)
    targets = np.asarray(targets).astype(np.int64).reshape(-1)
    domains = np.asarray(domains).astype(np.int64).reshape(-1)
    dcc = np.asarray(dcc_weights, dtype=np.float32)
    n = x.shape[0]

    logw = np.full_like(dcc, -np.inf)
    np.log(dcc, out=logw, where=dcc > 0)

    # host-side target-score sum: sum_i x[i, t_i] + logw[d_i, t_i] (f64 exact)
    t_scores = x[np.arange(n), targets].astype(np.float64)
    t_scores += logw.astype(np.float64)[domains, targets]
    tsum = float(t_scores.sum())

    # sort rows by domain into 512-row single-domain chunks
    order = np.argsort(domains, kind="stable")
    counts = np.bincount(domains, minlength=D)
    aligned = ((counts + CHUNK - 1) // CHUNK) * CHUNK
    total_chunks = n_cores * NCHUNK
    assert aligned.sum() <= total_chunks * CHUNK, "domain counts exceed capacity"
    base = np.concatenate(([0], np.cumsum(aligned)[:-1]))
    starts = np.concatenate(([0], np.cumsum(counts)[:-1]))
    dom_sorted = domains[order]
    rank = np.arange(n) - np.repeat(starts, counts)
    pos = base[dom_sorted] + rank  # padded position of each sorted row

    chunk_dom = np.full(total_chunks, D - 1, dtype=np.int64)
    for dd in range(D):
        c0 = base[dd] // CHUNK
        c1 = c0 + aligned[dd] // CHUNK
        chunk_dom[c0:c1] = dd

    # padded, clipped, sorted x in int8 (pad rows = 0)
    xi = np.clip(np.rint(x * S_INT8), -X_CLIP * S_INT8, X_CLIP * S_INT8).astype(
        np.int8
    )
    Xp = np.zeros((total_chunks * CHUNK, C), dtype=np.int8)
    Xp[pos] = xi[order]

    # pad correction: pad row in chunk i contributes
    #   ln( e_lo(i) * sum_{j<128} w_bf[d,j] + e_hi(i) * sum_{j>=128} w_bf[d,j] )
    # where e_* is the engine-exact exp(0): Schraudolph E0 on DVE, 1.0 on ACT.
    w_bf = dcc.astype(BF16_NP)
    E0 = float(_schraudolph_np(np.float32(0.0)))
    W_lo = w_bf[:, :C0].astype(np.float64).sum(axis=1)
    W_hi = w_bf[:, C0:].astype(np.float64).sum(axis=1)
    # chunk i -> tile t = i//CPT -> ops (2t, 2t+1)
    i_arr = np.arange(NCHUNK)  # chunk within core (same for every core)
    t_arr = i_arr // CPT
    e_lo = np.array(
        [1.0 if _op_engine(2 * t) == "act" else E0 for t in t_arr]
    )
    e_hi = np.array(
        [1.0 if _op_engine(2 * t + 1) == "act" else E0 for t in t_arr]
    )
    n_pad_per_chunk = np.full(total_chunks, CHUNK, dtype=np.int64)
    real_per_chunk = np.bincount(pos // CHUNK, minlength=total_chunks)
    n_pad_per_chunk -= real_per_chunk
    S_pad = (
        np.tile(e_lo, n_cores) * W_lo[chunk_dom]
        + np.tile(e_hi, n_cores) * W_hi[chunk_dom]
    )
    correction = float((n_pad_per_chunk * np.log(S_pad)).sum())

    in_maps = []
    for c in range(n_cores):
        blk = Xp[c * M : (c + 1) * M]  # [M, 200] i8
        x0 = np.ascontiguousarray(
            blk[:, :C0].reshape(NGROUP, GROUP_ROWS, C0).transpose(0, 2, 1)
        ).reshape(NGROUP * C0, GROUP_ROWS)
        x1 = np.ascontiguousarray(
            blk[:, C0:].reshape(NGROUP, GROUP_ROWS, C1).transpose(0, 2, 1)
        ).reshape(NGROUP * C1, GROUP_ROWS)
        cd = chunk_dom[c * NCHUNK : (c + 1) * NCHUNK]
        wt0 = np.zeros((C0, NCHUNK * 128), dtype=BF16_NP)
        wt1 = np.zeros((C1, NCHUNK * 128), dtype=BF16_NP)
        idx = np.arange(NCHUNK)
        wt0[:, idx * 128 + idx] = w_bf[cd, :C0].T
        wt1[:, idx * 128 + idx] = w_bf[cd, C0:].T
        in_maps.append({"x0": x0, "x1": x1, "wt0": wt0, "wt1": wt1})

    aux = {"tsum": tsum, "correction": correction, "n": n}
    return in_maps, aux


def combine(results, aux):
    """Combine per-core [128, 1] ln-S partials minus host-side terms."""
    total = -aux["tsum"] - aux["correction"]
    for r in results:
        o = np.asarray(r["out"], dtype=np.float64)
        total += float(o[:, 0].sum())
    return np.float32(total / aux["n"])


_PROGRAM_CACHE = {}


def _get_program(n_per, n_cores):
    key = (n_per, n_cores)
    if key not in _PROGRAM_CACHE:
        _PROGRAM_CACHE[key] = build_program(n_per, n_cores)
    return _PROGRAM_CACHE[key]


LAST_RESULT = None


def run(inputs, targets, domains, dcc_weights, trace=False, tmpdir=None):
    global LAST_RESULT
    n = inputs.shape[0]
    assert n % N_CORES == 0
    n_per = n // N_CORES
    (nc,) = _get_program(n_per, N_CORES)
    in_maps, aux = prep_inmaps(
        inputs, targets, domains, dcc_weights, N_CORES, n_per
    )
    res = run_bass_kernel_spmd(
        nc, in_maps, core_ids=list(range(N_CORES)), trace=trace, tmpdir=tmpdir
    )
    LAST_RESULT = res
    return combine(res.results, aux)


def kernel(inputs, targets, domains, dcc_weights):
    targets = np.asarray(targets).reshape(-1)
    domains_a = np.asarray(domains).reshape(-1)
    counts = np.bincount(domains_a, minlength=D) if domains_a.size else np.zeros(D)
    aligned_total = (((counts + CHUNK - 1) // CHUNK) * CHUNK).sum()
    if (
        np.any((targets < 0) | (targets >= C))
        or np.asarray(inputs).shape[0] != N_TOTAL
        or np.any(domains_a < 0)
        or np.any(domains_a >= D)
        or aligned_total > N_CORES * NCHUNK * CHUNK
    ):
        # exact but slow host fallback (never hit for the spec'd inputs)
        x = np.asarray(inputs, dtype=np.float64)
        dcc = np.asarray(dcc_weights, dtype=np.float64)
        logw = np.where(dcc > 0, np.log(np.maximum(dcc, 1e-300)), -np.inf)
        scores = logw[domains_a] + x
        m = scores.max(axis=1)
        lse = m + np.log(np.exp(scores - m[:, None]).sum(axis=1))
        tgt = np.clip(targets, 0, C - 1)
        ts = scores[np.arange(x.shape[0]), tgt]
        valid = targets != IGNORE
        return np.float32(np.where(valid, lse - ts, 0.0).sum() / x.shape[0])
    return run(inputs, targets, domains, dcc_weights, trace=False)
